# revision 92
# baseline (speedup 1.0000x reference)
"""DiffMLAAttention Trainium2 kernel, tensor-parallel over heads across 8 NeuronCores.

Per-core work (core c): 2 KV heads {2c, 2c+1}, 4 Q heads {4c..4c+3}.
Host folds the low-rank projections into effective weights:
    W_eff_q = W_DQ @ W_UQ,  W_eff_k = W_DKV @ W_UK,  W_eff_v = W_DKV @ W_UV
so the device computes q/k/v directly from x.  Weights are column-sharded by
head; W_out row-sharded; the host sums the 8 partial outputs (the row-parallel
all-reduce).

fp8 projection scheme (DoubleRow, 0.5 cycles/row):
  x is host-split into x = x_hi + x_lo, both fp8e4 at the same scale (the
  residual self-selects a smaller exponent, so one scale serves both and the
  DoubleRow slot-sum W0^T@X0 + W1^T@X1 needs no per-slot rescaling).
  - q/k projections: for F1_RAW of the 16 ko-tiles the weight is raw fp8
    (one DR inst per ko: W8 broadcast over both slots x (x_hi, x_lo) -> 2x
    PE speed; quantization noise of W8 is the only error source, measured
    1.76e-2 rel-l2 at F1_RAW=16, scaling ~sqrt(F1_RAW)).  Remaining kos use
    the two-term corrected weight (2 DR insts: W_hi then W_lo, each over
    (x_hi, x_lo)) at bf16 speed but better-than-bf16 accuracy.
  - v projection (+ lambda cols): always corrected (bf16 speed, ~no error).
  Descale 1/(SX*SW) is folded into the PSUM eviction (Act scale / DVE
  tensor_scalar imm).
Everything else runs fp16 (PE cost identical to bf16, DVE 2x mode intact,
~8x less quantization noise), PSUM fp32.

Schedule (cost-model-driven, 205.0us vs 214.1us for the bf16 predecessor):
half-block-staggered software pipeline over 256-column chunks. Iteration j
weaves proj(j) chains (k, q, then v) with block(j-1) h1 score/exp units
(early), block(j)'s h0 units (from gi>=3, rate 2) and PULLed h1 units (from
gi>=5, rate 1 — pulling more of the late blocks smooths the Act-bound
tail); block(j-1) combine -> DMA-XBAR transpose -> PV -> W_out (split into
1024-col halves, interleaved with leftover units) complete inside iter j.
The last block's combines run inside iter 7 so the epilogue is only
PV/W_out with quarter-granular evict+DMA.  Engine split: exp/sigmoid and
half the evicts on Act, rope muls + mask adds + combine + the other evicts
on DVE, k/v history flushes on GPSIMD(Pool) — Pool cannot touch PSUM and
its ~95ns-launch/0.4-0.6-efficiency ops are too slow for anything
latency-critical (measured: combine subs or rope shift copies on Pool
regress 10-60us).  DMA queues: weights+x-chunks on the scalar HWDGE queue,
pth transposes + first x chunks + last-block y on sync, wout/nblam + mid y
on Pool SWDGE; x8 is chunk-major so every DMA line is 512B contiguous
(256B lines cost 1.85x in the DMA model).
"""
import math

import numpy as np
import ml_dtypes

import concourse.bass as bass
import concourse.mybir as mybir
import concourse.tile as tile
from concourse import bacc
from concourse.masks import make_causal_mask

P = 128
L = 2048
D = 2048
DC = 512
DCQ = 1024
DH = 128
NH = 16
NCORES = 8
HL = NH // NCORES        # 2 local kv heads
QL = 2 * HL              # 4 local q heads
LT = L // P              # 16 q tiles
F32 = mybir.dt.float32
BF16 = mybir.dt.bfloat16
FP16 = mybir.dt.float16
FP8 = mybir.dt.float8e4
DR = mybir.MatmulPerfMode.DoubleRow
E4M3 = ml_dtypes.float8_e4m3
SCALE = 1.0 / math.sqrt(DH)

XC = 256                 # L-chunk width in projection phase
NC_CHUNKS = L // XC      # 8
SC = 1024                # score chunk width (2 fp32 psum banks)
HALF = DH // 2
NKO = D // P

F1_RAW = 12              # of the 16 ko-tiles: raw-fp8-weight (2x) count
SX = 32.0                # x fp8 scale
SWQ = 2048.0             # weffq fp8 scale
SWK = 2048.0
SWV = 2048.0
DESC_Q = 1.0 / (SX * SWQ)
DESC_K = 1.0 / (SX * SWK)
DESC_V = 1.0 / (SX * SWV)


DEFAULT_KNOBS = {
    "pull": {4: 2, 5: 2, 6: 6, 7: 8},
    "eu_gis": (0, 1, 2),      # gis that each take a 1/3 batch of eu
    "h0_gi": 3,               # first gi for h0 units
    "pull_gi": 5,             # first gi for pulled h1 units
    "h0_rate": 2,
    "pull_rate": 1,
}


def build_nc(knobs=None):
    kn = dict(DEFAULT_KNOBS)
    if knobs:
        kn.update(knobs)
    nc = bacc.Bacc("TRN2", target_bir_lowering=False)

    # chunk-major hi/lo interleave: contiguous 512B DMA lines per (d, chunk)
    x8T = nc.dram_tensor("x8T", [D, NC_CHUNKS, 2, XC], FP8,
                         kind="ExternalInput")
    weffq = nc.dram_tensor("weffq", [D, 2, QL * DH], FP8, kind="ExternalInput")
    weffk = nc.dram_tensor("weffk", [D, 2, HL * DH], FP8, kind="ExternalInput")
    weffv = nc.dram_tensor("weffv", [D, 2, HL * DH + HL], FP8,
                           kind="ExternalInput")
    nblam = nc.dram_tensor("nblam", [P, HL], F32, kind="ExternalInput")
    wout = nc.dram_tensor("wout", [HL * DH, D], FP16, kind="ExternalInput")
    cosT = nc.dram_tensor("cosT", [DH, L], FP16, kind="ExternalInput")
    sinTs = nc.dram_tensor("sinTs", [DH, L], FP16, kind="ExternalInput")
    y = nc.dram_tensor("y", [L, D], FP16, kind="ExternalOutput")

    x8T_r = x8T.rearrange("(ko ki) c t l -> ki ko c t l", ki=P)

    with tile.TileContext(nc) as tc:
        with (
            tc.tile_pool(name="const", bufs=1) as constp,
            tc.tile_pool(name="big", bufs=1) as bigp,
            tc.tile_pool(name="wa", bufs=1) as wa,
            tc.tile_pool(name="xa", bufs=2) as xa,
            tc.tile_pool(name="tmpa", bufs=2) as tmpa,
            tc.tile_pool(name="wo", bufs=1) as wop,
            tc.tile_pool(name="pp", bufs=2) as pp,
            tc.tile_pool(name="pp0", bufs=2) as pp0,
            tc.tile_pool(name="ptp", bufs=2) as ptp,
            tc.tile_pool(name="atp", bufs=2) as atp,
            tc.tile_pool(name="outp", bufs=2) as outp,
            tc.tile_pool(name="smp", bufs=2) as smp,
            tc.tile_pool(name="psum", bufs=2, space="PSUM") as psum,
        ):
            # ---- weight / const DMAs (scalar HWDGE queue, first-use
            # order: weffk, then weffq groups, rope tables, rest) ----
            weffk_sb = wa.tile([P, NKO, 2, HL * DH], FP8)
            weffk_r = weffk.rearrange("(ko ki) t m -> ki ko t m", ki=P)
            weffq_sb = wa.tile([P, NKO, 2, QL * DH], FP8)
            weffq_r = weffq.rearrange("(ko ki) t m -> ki ko t m", ki=P)

            def wfetch(sb, r, g):
                g0, g1 = 4 * g, 4 * g + 4
                if g1 <= F1_RAW:
                    # raw-fp8 kos only ever read the hi tier
                    nc.scalar.dma_start(sb[:, g0:g1, 0, :], r[:, g0:g1, 0, :])
                else:
                    nc.scalar.dma_start(sb[:, g0:g1], r[:, g0:g1])

            for g in range(4):
                wfetch(weffk_sb, weffk_r, g)
            wfetch(weffq_sb, weffq_r, 0)
            cos_sb = constp.tile([DH, L], FP16, name="cos_sb")
            nc.scalar.dma_start(cos_sb[:], cosT[:])
            sin_sb = constp.tile([DH, L], FP16, name="sin_sb")
            nc.scalar.dma_start(sin_sb[:], sinTs[:])
            for g in range(1, 4):
                wfetch(weffq_sb, weffq_r, g)
            nblam_sb = constp.tile([P, HL], F32, name="nblam_sb")
            wout_sb = wop.tile([P, HL, D], FP16)
            wout_r = wout.rearrange("(ho ki) n -> ki ho n", ki=P)

            def fetch_wout():
                # deferred: an eager SWDGE fetch at t=0 would occupy the
                # serial DMA device ahead of the startup-critical weights
                nc.gpsimd.dma_start(nblam_sb[:], nblam[:])
                nc.gpsimd.dma_start(wout_sb[:], wout_r[:])

            kT_sb = bigp.tile([P, HL, L], FP16)       # roped k^T hist [dh, h, l]
            v_sb = bigp.tile([P, LT, HL * DH], FP16)  # v hist    [l%P, lt, h*dh]
            lam_sb = bigp.tile([P, LT, HL], F32)      # sigmoid lambda

            # x chunk prefetch on the sync HWDGE queue; one chunk of
            # lookahead.  First two chunks quartered for a fast start.
            xt_tiles = {}

            def xt_fetch(j):
                if j >= NC_CHUNKS:
                    return
                xt = xa.tile([P, NKO, 2, XC], FP8, tag="xa")
                if j < 1:
                    for kg in range(4):
                        nc.sync.dma_start(
                            xt[:, kg * 4:(kg + 1) * 4],
                            x8T_r[:, kg * 4:(kg + 1) * 4, j],
                        )
                elif j == 1:
                    nc.sync.dma_start(xt[:, 0:8], x8T_r[:, 0:8, j])
                    nc.sync.dma_start(xt[:, 8:16], x8T_r[:, 8:16, j])
                else:
                    # scalar HWDGE queue: keep sync free for pth transposes
                    nc.scalar.dma_start(xt[:], x8T_r[:, :, j])
                xt_tiles[j] = xt

            xt_fetch(0)
            weffv_sb = wa.tile([P, NKO, 2, HL * DH + HL], FP8)
            weffv_r = weffv.rearrange("(ko ki) t m -> ki ko t m", ki=P)
            for g in range(4):
                nc.sync.dma_start(
                    weffv_sb[:, 4 * g:4 * g + 4], weffv_r[:, 4 * g:4 * g + 4])
            cmask = constp.tile([P, P], F32)
            make_causal_mask(nc, cmask[:], mask_val=-1e9)

            def rope_evict(ps, out_ap, sl, tag, desc, late=False):
                """out = (desc*ps)*cos[:, sl] + shift(desc*ps)*sinTs[:, sl].

                Evict psum -> fp16 with the fp8 descale folded in; then
                GPSIMD: two partition-shifted copies; DVE: two aligned fp16
                muls + add (2x mode).
                """
                w = sl.stop - sl.start
                t = tmpa.tile([P, XC], FP16, tag=f"t{tag}")
                if late:
                    nc.vector.tensor_scalar_mul(t[:, :w], ps, desc)
                else:
                    nc.scalar.activation(
                        t[:, :w], ps, mybir.ActivationFunctionType.Copy,
                        scale=desc)
                rot = tmpa.tile([P, XC], FP16, tag=f"r{tag}")
                nc.vector.tensor_copy(rot[0:HALF, :w], t[HALF:DH, :w])
                nc.vector.tensor_copy(rot[HALF:DH, :w], t[0:HALF, :w])
                nc.vector.tensor_mul(rot[:, :w], rot[:, :w], sin_sb[:, sl])
                nc.vector.tensor_mul(t[:, :w], t[:, :w], cos_sb[:, sl])
                nc.vector.tensor_add(out_ap, t[:, :w], rot[:, :w])

            # ---- projection groups for chunk j (each a closure) ----
            qt_rings = {}
            kv_rings = {}

            def proj_chain(ps_ap, wsb, cols, xt, n_free):
                """Emit the fp8 DR chain: raw kos (W8-hi broadcast over the
                (x_hi, x_lo) slot pair), then corrected kos (W_hi, W_lo)."""
                last = (NKO - 1, 1 if F1_RAW < NKO else 0)
                first = True
                for ko in range(NKO):
                    xpair = xt[:, ko, :, :]
                    if ko < F1_RAW:
                        wslot = wsb[:, ko, 0:1, cols].broadcast_to(
                            [P, 2, cols.stop - cols.start])
                        nc.tensor.matmul(
                            ps_ap, wslot, xpair,
                            start=first,
                            stop=(ko == NKO - 1 and F1_RAW == NKO),
                            perf_mode=DR)
                        first = False
                    else:
                        for t in range(2):
                            wslot = wsb[:, ko, t:t + 1, cols].broadcast_to(
                                [P, 2, cols.stop - cols.start])
                            nc.tensor.matmul(
                                ps_ap, wslot, xpair,
                                start=first,
                                stop=(ko == NKO - 1 and t == 1),
                                perf_mode=DR)
                            first = False

            def proj_groups(j):
                sl = slice(j * XC, (j + 1) * XC)
                xt = xt_tiles[j]
                qt_rings[j] = qT = pp0.tile([P, QL, XC], FP16, tag="qt",
                                            name="qTr")
                kR = pp0.tile([P, HL, XC], FP16, tag="kr", name="kRr")
                vR = pp0.tile([P, XC // P, HL * DH], FP16, tag="vr", name="vRr")
                kv_rings[j] = (kR, vR)
                groups = []

                def kq_group(qh, kq):
                    def emit():
                        ps = psum.tile([P, XC], F32, tag="aqk", bufs=2)
                        wsb = weffk_sb if kq == "k" else weffq_sb
                        proj_chain(ps[:], wsb,
                                   slice(qh * P, (qh + 1) * P), xt, XC)
                        if kq == "k":
                            rope_evict(ps[:], kR[:, qh, :], sl, kq, DESC_K)
                        else:
                            rope_evict(ps[:], qT[:, qh, :], sl, kq, DESC_Q)
                    return emit

                def v_group(ls):
                    def emit():
                        lt_idx = j * (XC // P) + ls
                        psv = psum.tile([P, 512], F32, tag="sm", bufs=2)
                        nv = HL * DH + HL
                        first = True
                        for ko in range(NKO):
                            xpair = xt[:, ko, :, ls * P:(ls + 1) * P]
                            for t in range(2):
                                wslot = weffv_sb[:, ko, t:t + 1, :].broadcast_to(
                                    [P, 2, nv])
                                nc.tensor.matmul(
                                    psv[:, :nv], xpair, wslot,
                                    start=first,
                                    stop=(ko == NKO - 1 and t == 1),
                                    perf_mode=DR)
                                first = False
                        if j >= 5:
                            nc.vector.tensor_scalar_mul(
                                vR[:, ls, :], psv[:, :HL * DH], DESC_V)
                        else:
                            nc.scalar.activation(
                                vR[:, ls, :], psv[:, :HL * DH],
                                mybir.ActivationFunctionType.Copy,
                                scale=DESC_V)
                        # lambda = sigmoid(v-chain cols [256:258] + blam)
                        e = tmpa.tile([P, HL], F32, tag="sig")
                        for hh in range(HL):
                            nc.scalar.activation(
                                e[:, hh:hh + 1],
                                psv[:, HL * DH + hh:HL * DH + hh + 1],
                                mybir.ActivationFunctionType.Exp,
                                scale=-DESC_V,
                                bias=nblam_sb[:, hh:hh + 1],
                            )
                        nc.vector.tensor_scalar_add(e[:], e[:], 1.0)
                        nc.vector.reciprocal(lam_sb[:, lt_idx, :], e[:])
                    return emit

                for h in range(HL):
                    groups.append(kq_group(h, "k"))
                for qh in range(QL):
                    groups.append(kq_group(qh, "q"))
                for ls in range(XC // P):
                    groups.append(v_group(ls))
                return groups

            def flush_k(j):
                kR, _ = kv_rings[j]
                sl = slice(j * XC, (j + 1) * XC)
                nc.vector.tensor_copy(kT_sb[:, :, sl], kR[:])

            def flush_v(j):
                _, vR = kv_rings.pop(j)
                nc.gpsimd.tensor_copy(
                    v_sb[:, 2 * j:2 * j + 2, :], vR[:])

            # Cross-iteration state: per j-block prob tiles, sums, attnT, pth.
            blk = {}

            # ---- score units for j-block: one closure per (h,p_,qi,ck) ----
            def score_units(j):
                qa, qb = 2 * j, 2 * j + 1
                ptiles, sums = {}, {}
                blk[j] = {"pt": ptiles, "sm": sums}
                units = []

                qT = qt_rings[j]

                def unit(h, p_, qi, qt, ck, nck):
                    def emit():
                        qh = 2 * h + p_
                        lk = (qt + 1) * P
                        if ck == 0:
                            ptiles[(h, p_, qi)] = pp.tile(
                                [P, L], FP16, tag=f"P{h}{p_}{qi}",
                                name=f"P{h}{p_}{qi}")
                            sums[(h, p_, qi)] = (
                                smp.tile([P, 2], F32, tag=f"s{h}{p_}{qi}",
                                         name=f"s{h}{p_}{qi}"), nck)
                        ptile = ptiles[(h, p_, qi)]
                        sm = sums[(h, p_, qi)][0]
                        cw = min(SC, lk - ck * SC)
                        sps = psum.tile([P, SC], F32, tag="sps", bufs=2)
                        for sb0 in range(0, cw, 512):
                            sw = min(512, cw - sb0)
                            nc.tensor.matmul(
                                sps[:, sb0:sb0 + sw],
                                qT[:, qh, (qt % 2) * P:(qt % 2 + 1) * P],
                                kT_sb[:, h, ck * SC + sb0:ck * SC + sb0 + sw],
                                start=True,
                                stop=True,
                            )
                        if ck * SC <= qt * P < ck * SC + cw:
                            off = qt * P - ck * SC
                            nc.vector.tensor_add(
                                sps[:, off:off + P], sps[:, off:off + P],
                                cmask[:])
                        nc.scalar.activation(
                            ptile[:, ck * SC:ck * SC + cw],
                            sps[:, :cw],
                            mybir.ActivationFunctionType.Exp,
                            scale=SCALE,
                            accum_out=sm[:, ck:ck + 1],
                        )
                    return emit

                h0_units = []
                h1_units = []
                for h in range(HL):
                    for qi, qt in enumerate((qa, qb)):
                        for p_ in range(2):
                            nck = ((qt + 1) * P + SC - 1) // SC
                            for ck in range(nck):
                                (h0_units if h == 0 else h1_units).append(
                                    unit(h, p_, qi, qt, ck, nck))
                return h0_units, h1_units

            # ---- combine + transpose for (j, h): DVE + sync queue ----
            def combine_tp(j, h, only_qi=None):
                qa, qb = 2 * j, 2 * j + 1
                ptiles, sums = blk[j]["pt"], blk[j]["sm"]
                for qi, qt in enumerate((qa, qb)):
                    if only_qi is not None and qi != only_qi:
                        continue
                    lk = (qt + 1) * P
                    invs = []
                    for p_ in range(2):
                        sm, nck = sums[(h, p_, qi)]
                        if nck > 1:
                            r = smp.tile([P, 1], F32, tag=f"r{h}{p_}{qi}")
                            nc.vector.reduce_sum(
                                r[:], sm[:, :nck], axis=mybir.AxisListType.X)
                        else:
                            r = sm[:, 0:1]
                        inv = smp.tile([P, 1], F32, tag=f"i{h}{p_}{qi}")
                        nc.vector.reciprocal(inv[:], r[:])
                        invs.append(inv)
                    s2 = smp.tile([P, 1], F32, tag=f"l{h}{qi}")
                    nc.vector.tensor_mul(
                        s2[:], invs[1][:], lam_sb[:, qt, h:h + 1])
                    p0, p1 = ptiles[(h, 0, qi)], ptiles[(h, 1, qi)]
                    nc.vector.tensor_scalar_mul(p1[:, :lk], p1[:, :lk], s2[:])
                    nc.vector.tensor_scalar_mul(p0[:, :lk], p0[:, :lk], invs[0][:])
                    nc.vector.tensor_sub(p0[:, :lk], p0[:, :lk], p1[:, :lk])
                if only_qi in (None, 0):
                    pth = ptp.tile([P, LT, 2 * P], FP16, tag="pt", name="pth")
                    blk[j][("pth", h)] = pth
                pth = blk[j][("pth", h)]
                for qi, qt in enumerate((qa, qb)):
                    if only_qi is not None and qi != only_qi:
                        continue
                    for c0 in range(0, qt + 1, 8):
                        c1 = min(c0 + 8, qt + 1)
                        nc.sync.dma_start_transpose(
                            pth[:, c0:c1, qi * P:(qi + 1) * P],
                            ptiles[(h, 0, qi)][:, c0 * P:c1 * P],
                        )

            # ---- PV for (j, h): PE + Act attnT evict ----
            def pv_h(j, h, only_qi=None):
                qa, qb = 2 * j, 2 * j + 1
                if "at" not in blk[j]:
                    blk[j]["at"] = atp.tile([P, HL, 2 * P], FP16, tag="at",
                                            name="attnT")
                attnT = blk[j]["at"]
                if only_qi is None:
                    pth = blk[j].pop(("pth", h))
                    pv = psum.tile([P, 512], F32, tag="sm", bufs=2)
                    for kc in range(qa + 1):
                        nc.tensor.matmul(
                            pv[:, :2 * P],
                            v_sb[:, kc, h * DH:(h + 1) * DH],
                            pth[:, kc, :],
                            start=(kc == 0),
                            stop=False,
                        )
                    nc.tensor.matmul(
                        pv[:, P:2 * P],
                        v_sb[:, qb, h * DH:(h + 1) * DH],
                        pth[:, qb, P:2 * P],
                        start=False,
                        stop=True,
                    )
                    nc.vector.tensor_copy(attnT[:, h, :], pv[:, :2 * P])
                else:
                    qt = (qa, qb)[only_qi]
                    pth = blk[j][("pth", h)]
                    if only_qi == 1:
                        blk[j].pop(("pth", h))
                    pv = psum.tile([P, 512], F32, tag="sm", bufs=2)
                    for kc in range(qt + 1):
                        nc.tensor.matmul(
                            pv[:, :P],
                            v_sb[:, kc, h * DH:(h + 1) * DH],
                            pth[:, kc, only_qi * P:(only_qi + 1) * P],
                            start=(kc == 0),
                            stop=(kc == qt),
                        )
                    nc.vector.tensor_copy(
                        attnT[:, h, only_qi * P:(only_qi + 1) * P], pv[:, :P])

            # ---- W_out for block j, one qt, one 1024-half: PE + evict ----
            def wout_half(j, qi, half):
                qa, qb = 2 * j, 2 * j + 1
                qt = (qa, qb)[qi]
                attnT = blk[j]["at"]
                key = ("osb", qi)
                if key not in blk[j]:
                    blk[j][key] = outp.tile([P, D], FP16, tag=f"osb{qi}",
                                            name=f"osb{qi}")
                osb = blk[j][key]
                last = j == NC_CHUNKS - 1
                po = psum.tile([P, SC], F32, tag="sps", bufs=2)
                for nb in range(2):
                    for h in range(HL):
                        nc.tensor.matmul(
                            po[:, nb * 512:(nb + 1) * 512],
                            attnT[:, h, qi * P:(qi + 1) * P],
                            wout_sb[:, h,
                                    (2 * half + nb) * 512:
                                    (2 * half + nb + 1) * 512],
                            start=(h == 0),
                            stop=(h == HL - 1),
                        )
                    if last:
                        # quarter-granular evict+DMA: short drain
                        c0 = (2 * half + nb) * 512
                        if nb == 0:
                            nc.vector.tensor_copy(
                                osb[:, c0:c0 + 512], po[:, :512])
                        else:
                            nc.scalar.activation(
                                osb[:, c0:c0 + 512], po[:, 512:],
                                mybir.ActivationFunctionType.Copy)
                        nc.sync.dma_start(
                            y[qt * P:(qt + 1) * P, c0:c0 + 512],
                            osb[:, c0:c0 + 512])
                if not last:
                    if half == 0:
                        nc.vector.tensor_copy(
                            osb[:, half * SC:(half + 1) * SC], po[:])
                    else:
                        nc.scalar.activation(
                            osb[:, half * SC:(half + 1) * SC], po[:],
                            mybir.ActivationFunctionType.Copy)
                        nc.gpsimd.dma_start(
                            y[qt * P:(qt + 1) * P, :], osb[:])
                        blk[j].pop(key)

            # ---- half-block staggered emission ----
            # iter j: proj(j) woven with block(j-1) h1 scores (early) and
            # block(j) h0 scores (late); block(j-1) combine/tp/PV/W_out all
            # complete within iter j.
            # h1-pull: how many of block j's h1 units run inside iter j
            # (after its q2/q3 ropes) instead of deferring to iter j+1 —
            # smooths the Act-bound tail.
            PULL = kn["pull"]
            RATIO = kn.get("ratio", 1.0)
            pend_h1 = {}
            for j in range(NC_CHUNKS + 1):
                a = j if j < NC_CHUNKS else None
                s = j - 1 if j >= 1 else None          # writeback block

                if a is not None and j > 0:
                    xt_fetch(j + 1)
                pg = proj_groups(a) if a is not None else []
                pulled = []
                if a is not None:
                    h0u, h1u = score_units(a)
                    if j == 0:
                        # chunk 0: weights/tables still streaming in; defer
                        # everything to iter 1
                        pend_h1[a] = h0u + h1u
                        h0u = []
                    else:
                        np_ = PULL.get(a, 0)
                        pulled = h1u[:np_]
                        pend_h1[a] = h1u[np_:]
                else:
                    h0u = []
                eu = pend_h1.pop(s, []) if s is not None else []

                stream = []
                state = {"e": 0, "l": 0, "p": 0}

                def take(lst, key, k):
                    for _ in range(k):
                        if state[key] < len(lst):
                            stream.append(lst[state[key]])
                            state[key] += 1

                npg = len(pg)
                # pg order: k0 k1 q0 q1 q2 q3 v0 v1
                for gi in range(max(npg, 1)):
                    if gi == (4 if s == 0 else 2) and s is not None:
                        stream.append(lambda s=s: combine_tp(s, 0))
                    if pg:
                        stream.append(pg.pop(0))
                    if gi == 3 and a is not None:
                        stream.append(lambda a=a: flush_k(a))
                    if gi in kn["eu_gis"]:
                        take(eu, "e",
                             (len(eu) + len(kn["eu_gis"]) - 1)
                             // len(kn["eu_gis"]))
                    if gi == 7 and s is not None:
                        take(eu, "e", len(eu))
                        stream.append(lambda s=s: pv_h(s, 0))
                    if gi >= kn["h0_gi"] and j > 0:
                        # h0 units read only the q0/q1 ropes (pg2/pg3)
                        take(h0u, "l", kn["h0_rate"])
                    if gi >= kn["pull_gi"]:
                        # pulled h1 units read the q2/q3 ropes (pg4/pg5)
                        take(pulled, "p", kn["pull_rate"])
                    if gi == 6 and s is not None:
                        stream.append(lambda s=s: combine_tp(s, 1))
                if s is not None:
                    if npg == 0:
                        # epilogue: combine(s,0,*) and (s,1,0) ran in iter s;
                        # pv(1,0) late — its transposes are last in the queue
                        stream.append(lambda s=s: pv_h(s, 0, 0))
                        stream.append(lambda s=s: pv_h(s, 0, 1))
                        stream.append(lambda s=s: combine_tp(s, 1, 1))
                        stream.append(lambda s=s: pv_h(s, 1, 0))
                        stream.append(lambda s=s: wout_half(s, 0, 0))
                        stream.append(lambda s=s: wout_half(s, 0, 1))
                        stream.append(lambda s=s: pv_h(s, 1, 1))
                        stream.append(lambda s=s: wout_half(s, 1, 0))
                        stream.append(lambda s=s: wout_half(s, 1, 1))
                    else:
                        take(eu, "e", len(eu))
                        stream.append(lambda s=s: pv_h(s, 1))
                        take(h0u, "l", 2)
                        stream.append(lambda s=s: wout_half(s, 0, 0))
                        take(h0u, "l", 1)
                        stream.append(lambda s=s: wout_half(s, 0, 1))
                        take(h0u, "l", 1)
                        stream.append(lambda s=s: wout_half(s, 1, 0))
                        take(h0u, "l", 1)
                        stream.append(lambda s=s: wout_half(s, 1, 1))
                take(h0u, "l", len(h0u))
                take(pulled, "p", len(pulled))
                if a is not None:
                    stream.append(lambda a=a: flush_v(a))
                if a == NC_CHUNKS - 1:
                    # start the last block's combines + transposes now so
                    # the epilogue's PV has its pth ready
                    stream.append(lambda a=a: combine_tp(a, 0, 0))
                    stream.append(lambda a=a: combine_tp(a, 0, 1))
                    stream.append(lambda a=a: combine_tp(a, 1, 0))
                if j == 0:
                    stream.insert(len(stream) * 3 // 4, lambda: xt_fetch(1))
                    stream.insert(len(stream) // 2, fetch_wout)

                for emit in stream:
                    emit()
                if a is not None:
                    xt_tiles.pop(a, None)

    nc.compile()
    return nc


_NC = None


def _get_nc():
    global _NC
    if _NC is None:
        _NC = build_nc()
    return _NC


def _rope_tables():
    inv_freq = 1.0 / (10000.0 ** (np.arange(0, DH, 2, dtype=np.float32) / DH))
    t = np.arange(L, dtype=np.float32)
    freqs = np.outer(t, inv_freq)                    # [L, DH/2]
    emb = np.concatenate([freqs, freqs], axis=-1)    # [L, DH]
    cos = np.cos(emb).astype(np.float32)
    sin = np.sin(emb).astype(np.float32)
    sign = np.where(np.arange(DH) < DH // 2, -1.0, 1.0).astype(np.float32)
    cosT = np.ascontiguousarray(cos.T)               # [DH, L]
    sinTs = np.ascontiguousarray(sin.T * sign[:, None])
    return cosT, sinTs


def _hp(a):
    return np.ascontiguousarray(np.asarray(a, dtype=np.float32)).astype(
        np.float16
    )


def _fp8_pair(a, scale):
    """[..., n] -> [..., 2, n]: hi = e4m3(a*scale), lo = e4m3(a*scale - hi)."""
    s = np.asarray(a, dtype=np.float32) * scale
    hi = s.astype(E4M3)
    lo = (s - hi.astype(np.float32)).astype(E4M3)
    return np.ascontiguousarray(np.stack([hi, lo], axis=-2))


def prepare_in_maps(x, W_DKV, W_UK, W_UV, W_DQ, W_UQ, W_lam, b_lam, W_out):
    x = np.asarray(x, dtype=np.float32)
    W_DKV = np.asarray(W_DKV, dtype=np.float32)
    W_UK = np.asarray(W_UK, dtype=np.float32)
    W_UV = np.asarray(W_UV, dtype=np.float32)
    W_DQ = np.asarray(W_DQ, dtype=np.float32)
    W_UQ = np.asarray(W_UQ, dtype=np.float32)
    W_lam = np.asarray(W_lam, dtype=np.float32)
    b_lam = np.asarray(b_lam, dtype=np.float32)
    W_out = np.asarray(W_out, dtype=np.float32)

    # Host-side low-rank fold (fp32 BLAS)
    Weffq = W_DQ @ W_UQ                              # [D, 2*NH*DH]
    Weffk = W_DKV @ W_UK                             # [D, NH*DH]
    Weffv = W_DKV @ W_UV

    x8 = _fp8_pair(x[0].T, SX)                       # [D, 2, L]
    x8 = np.ascontiguousarray(                       # [D, NC, 2, XC]
        x8.reshape(D, 2, NC_CHUNKS, XC).transpose(0, 2, 1, 3))
    cosT, sinTs = _rope_tables()
    cosT_hp, sinTs_hp = _hp(cosT), _hp(sinTs)

    in_maps = []
    for c in range(NCORES):
        nblam_ = np.ascontiguousarray(
            np.broadcast_to(-b_lam[2 * c:2 * c + 2][None, :], (P, HL))
        ).astype(np.float32)
        in_maps.append({
            "x8T": x8,
            "weffq": _fp8_pair(Weffq[:, 4 * c * 128:(4 * c + 4) * 128], SWQ),
            "weffk": _fp8_pair(Weffk[:, c * 256:(c + 1) * 256], SWK),
            "weffv": _fp8_pair(np.concatenate(
                [Weffv[:, c * 256:(c + 1) * 256],
                 W_lam[:, 2 * c:2 * c + 2]], axis=1), SWV),
            "nblam": nblam_,
            "wout": _hp(W_out[c * 256:(c + 1) * 256, :]),
            "cosT": cosT_hp,
            "sinTs": sinTs_hp,
        })
    return in_maps


def kernel(x, W_DKV, W_UK, W_UV, W_DQ, W_UQ, W_lam, b_lam, W_out):
    in_maps = prepare_in_maps(
        x, W_DKV, W_UK, W_UV, W_DQ, W_UQ, W_lam, b_lam, W_out)

    from concourse.bass_utils import run_bass_kernel_spmd
    nc = _get_nc()
    res = run_bass_kernel_spmd(nc, in_maps, core_ids=list(range(NCORES)))
    y = np.zeros((L, D), dtype=np.float32)
    for c in range(NCORES):
        y += np.asarray(res.results[c]["y"], dtype=np.float32)
    return y.reshape(1, L, D)


# revision 93
# speedup vs baseline: 1.0005x; 1.0005x over previous
"""DiffMLAAttention Trainium2 kernel, tensor-parallel over heads across 8 NeuronCores.

Per-core work (core c): 2 KV heads {2c, 2c+1}, 4 Q heads {4c..4c+3}.
Host folds the low-rank projections into effective weights:
    W_eff_q = W_DQ @ W_UQ,  W_eff_k = W_DKV @ W_UK,  W_eff_v = W_DKV @ W_UV
so the device computes q/k/v directly from x.  Weights are column-sharded by
head; W_out row-sharded; the host sums the 8 partial outputs (the row-parallel
all-reduce).

fp8 projection scheme (DoubleRow, 0.5 cycles/row):
  x is host-split into x = x_hi + x_lo, both fp8e4 at the same scale (the
  residual self-selects a smaller exponent, so one scale serves both and the
  DoubleRow slot-sum W0^T@X0 + W1^T@X1 needs no per-slot rescaling).
  - q/k projections: for F1_RAW of the 16 ko-tiles the weight is raw fp8
    (one DR inst per ko: W8 broadcast over both slots x (x_hi, x_lo) -> 2x
    PE speed; quantization noise of W8 is the only error source, measured
    1.76e-2 rel-l2 at F1_RAW=16, scaling ~sqrt(F1_RAW)).  Remaining kos use
    the two-term corrected weight (2 DR insts: W_hi then W_lo, each over
    (x_hi, x_lo)) at bf16 speed but better-than-bf16 accuracy.
  - v projection (+ lambda cols): always corrected (bf16 speed, ~no error).
  Descale 1/(SX*SW) is folded into the PSUM eviction (Act scale / DVE
  tensor_scalar imm).
Everything else runs fp16 (PE cost identical to bf16, DVE 2x mode intact,
~8x less quantization noise), PSUM fp32.

Schedule (cost-model-driven, 205.0us vs 214.1us for the bf16 predecessor):
half-block-staggered software pipeline over 256-column chunks. Iteration j
weaves proj(j) chains (k, q, then v) with block(j-1) h1 score/exp units
(early), block(j)'s h0 units (from gi>=3, rate 2) and PULLed h1 units (from
gi>=5, rate 1 — pulling more of the late blocks smooths the Act-bound
tail); block(j-1) combine -> DMA-XBAR transpose -> PV -> W_out (split into
1024-col halves, interleaved with leftover units) complete inside iter j.
The last block's combines run inside iter 7 so the epilogue is only
PV/W_out with quarter-granular evict+DMA.  Engine split: exp/sigmoid and
half the evicts on Act, rope muls + mask adds + combine + the other evicts
on DVE, k/v history flushes on GPSIMD(Pool) — Pool cannot touch PSUM and
its ~95ns-launch/0.4-0.6-efficiency ops are too slow for anything
latency-critical (measured: combine subs or rope shift copies on Pool
regress 10-60us).  DMA queues: weights+x-chunks on the scalar HWDGE queue,
pth transposes + first x chunks + last-block y on sync, wout/nblam + mid y
on Pool SWDGE; x8 is chunk-major so every DMA line is 512B contiguous
(256B lines cost 1.85x in the DMA model).
"""
import math

import numpy as np
import ml_dtypes

import concourse.bass as bass
import concourse.mybir as mybir
import concourse.tile as tile
from concourse import bacc
from concourse.masks import make_causal_mask

P = 128
L = 2048
D = 2048
DC = 512
DCQ = 1024
DH = 128
NH = 16
NCORES = 8
HL = NH // NCORES        # 2 local kv heads
QL = 2 * HL              # 4 local q heads
LT = L // P              # 16 q tiles
F32 = mybir.dt.float32
BF16 = mybir.dt.bfloat16
FP16 = mybir.dt.float16
FP8 = mybir.dt.float8e4
DR = mybir.MatmulPerfMode.DoubleRow
E4M3 = ml_dtypes.float8_e4m3
SCALE = 1.0 / math.sqrt(DH)

XC = 256                 # L-chunk width in projection phase
NC_CHUNKS = L // XC      # 8
SC = 1024                # score chunk width (2 fp32 psum banks)
HALF = DH // 2
NKO = D // P

F1_RAW = 12              # of the 16 ko-tiles: raw-fp8-weight (2x) count
SX = 32.0                # x fp8 scale
SWQ = 2048.0             # weffq fp8 scale
SWK = 2048.0
SWV = 2048.0
DESC_Q = 1.0 / (SX * SWQ)
DESC_K = 1.0 / (SX * SWK)
DESC_V = 1.0 / (SX * SWV)


DEFAULT_KNOBS = {
    "pull": {4: 2, 5: 2, 6: 6, 7: 8},
    "eu_gis": (0, 1, 2),      # gis that each take a 1/3 batch of eu
    "h0_gi": 3,               # first gi for h0 units
    "pull_gi": 5,             # first gi for pulled h1 units
    "h0_rate": 2,
    "pull_rate": 1,
}


def build_nc(knobs=None):
    kn = dict(DEFAULT_KNOBS)
    if knobs:
        kn.update(knobs)
    nc = bacc.Bacc("TRN2", target_bir_lowering=False)

    # chunk-major hi/lo interleave: contiguous 512B DMA lines per (d, chunk)
    x8T = nc.dram_tensor("x8T", [D, NC_CHUNKS, 2, XC], FP8,
                         kind="ExternalInput")
    weffq = nc.dram_tensor("weffq", [D, 2, QL * DH], FP8, kind="ExternalInput")
    weffk = nc.dram_tensor("weffk", [D, 2, HL * DH], FP8, kind="ExternalInput")
    weffv = nc.dram_tensor("weffv", [D, 2, HL * DH + HL], FP8,
                           kind="ExternalInput")
    nblam = nc.dram_tensor("nblam", [P, HL], F32, kind="ExternalInput")
    wout = nc.dram_tensor("wout", [HL * DH, D], FP16, kind="ExternalInput")
    cosT = nc.dram_tensor("cosT", [DH, L], FP16, kind="ExternalInput")
    sinTs = nc.dram_tensor("sinTs", [DH, L], FP16, kind="ExternalInput")
    y = nc.dram_tensor("y", [L, D], FP16, kind="ExternalOutput")

    x8T_r = x8T.rearrange("(ko ki) c t l -> ki ko c t l", ki=P)

    with tile.TileContext(nc) as tc:
        with (
            tc.tile_pool(name="const", bufs=1) as constp,
            tc.tile_pool(name="big", bufs=1) as bigp,
            tc.tile_pool(name="wa", bufs=1) as wa,
            tc.tile_pool(name="xa", bufs=2) as xa,
            tc.tile_pool(name="tmpa", bufs=2) as tmpa,
            tc.tile_pool(name="wo", bufs=1) as wop,
            tc.tile_pool(name="pp", bufs=2) as pp,
            tc.tile_pool(name="pp0", bufs=2) as pp0,
            tc.tile_pool(name="ptp", bufs=3) as ptp,
            tc.tile_pool(name="atp", bufs=2) as atp,
            tc.tile_pool(name="outp", bufs=2) as outp,
            tc.tile_pool(name="smp", bufs=2) as smp,
            tc.tile_pool(name="psum", bufs=2, space="PSUM") as psum,
        ):
            # ---- weight / const DMAs (scalar HWDGE queue, first-use
            # order: weffk, then weffq groups, rope tables, rest) ----
            weffk_sb = wa.tile([P, NKO, 2, HL * DH], FP8)
            weffk_r = weffk.rearrange("(ko ki) t m -> ki ko t m", ki=P)
            weffq_sb = wa.tile([P, NKO, 2, QL * DH], FP8)
            weffq_r = weffq.rearrange("(ko ki) t m -> ki ko t m", ki=P)

            def wfetch(sb, r, g):
                g0, g1 = 4 * g, 4 * g + 4
                if g1 <= F1_RAW:
                    # raw-fp8 kos only ever read the hi tier
                    nc.scalar.dma_start(sb[:, g0:g1, 0, :], r[:, g0:g1, 0, :])
                else:
                    nc.scalar.dma_start(sb[:, g0:g1], r[:, g0:g1])

            for g in range(4):
                wfetch(weffk_sb, weffk_r, g)
            wfetch(weffq_sb, weffq_r, 0)
            cos_sb = constp.tile([DH, L], FP16, name="cos_sb")
            nc.scalar.dma_start(cos_sb[:], cosT[:])
            sin_sb = constp.tile([DH, L], FP16, name="sin_sb")
            nc.scalar.dma_start(sin_sb[:], sinTs[:])
            for g in range(1, 4):
                wfetch(weffq_sb, weffq_r, g)
            nblam_sb = constp.tile([P, HL], F32, name="nblam_sb")
            wout_sb = wop.tile([P, HL, D], FP16)
            wout_r = wout.rearrange("(ho ki) n -> ki ho n", ki=P)

            def fetch_wout():
                # deferred: an eager SWDGE fetch at t=0 would occupy the
                # serial DMA device ahead of the startup-critical weights
                nc.gpsimd.dma_start(nblam_sb[:], nblam[:])
                nc.gpsimd.dma_start(wout_sb[:], wout_r[:])

            kT_sb = bigp.tile([P, HL, L], FP16)       # roped k^T hist [dh, h, l]
            v_sb = bigp.tile([P, LT, HL * DH], FP16)  # v hist    [l%P, lt, h*dh]
            lam_sb = bigp.tile([P, LT, HL], F32)      # sigmoid lambda

            # x chunk prefetch on the sync HWDGE queue; one chunk of
            # lookahead.  First two chunks quartered for a fast start.
            xt_tiles = {}

            def xt_fetch(j):
                if j >= NC_CHUNKS:
                    return
                xt = xa.tile([P, NKO, 2, XC], FP8, tag="xa")
                if j < 1:
                    for kg in range(4):
                        nc.sync.dma_start(
                            xt[:, kg * 4:(kg + 1) * 4],
                            x8T_r[:, kg * 4:(kg + 1) * 4, j],
                        )
                elif j == 1:
                    nc.sync.dma_start(xt[:, 0:8], x8T_r[:, 0:8, j])
                    nc.sync.dma_start(xt[:, 8:16], x8T_r[:, 8:16, j])
                else:
                    # scalar HWDGE queue: keep sync free for pth transposes
                    nc.scalar.dma_start(xt[:], x8T_r[:, :, j])
                xt_tiles[j] = xt

            xt_fetch(0)
            weffv_sb = wa.tile([P, NKO, 2, HL * DH + HL], FP8)
            weffv_r = weffv.rearrange("(ko ki) t m -> ki ko t m", ki=P)
            for g in range(4):
                nc.sync.dma_start(
                    weffv_sb[:, 4 * g:4 * g + 4], weffv_r[:, 4 * g:4 * g + 4])
            cmask = constp.tile([P, P], F32)
            make_causal_mask(nc, cmask[:], mask_val=-1e9)

            def rope_evict(ps, out_ap, sl, tag, desc, late=False):
                """out = (desc*ps)*cos[:, sl] + shift(desc*ps)*sinTs[:, sl].

                Evict psum -> fp16 with the fp8 descale folded in; then
                GPSIMD: two partition-shifted copies; DVE: two aligned fp16
                muls + add (2x mode).
                """
                w = sl.stop - sl.start
                t = tmpa.tile([P, XC], FP16, tag=f"t{tag}")
                if late:
                    nc.vector.tensor_scalar_mul(t[:, :w], ps, desc)
                else:
                    nc.scalar.activation(
                        t[:, :w], ps, mybir.ActivationFunctionType.Copy,
                        scale=desc)
                rot = tmpa.tile([P, XC], FP16, tag=f"r{tag}")
                nc.vector.tensor_copy(rot[0:HALF, :w], t[HALF:DH, :w])
                nc.vector.tensor_copy(rot[HALF:DH, :w], t[0:HALF, :w])
                nc.vector.tensor_mul(rot[:, :w], rot[:, :w], sin_sb[:, sl])
                nc.vector.tensor_mul(t[:, :w], t[:, :w], cos_sb[:, sl])
                nc.vector.tensor_add(out_ap, t[:, :w], rot[:, :w])

            # ---- projection groups for chunk j (each a closure) ----
            qt_rings = {}
            kv_rings = {}

            def proj_chain(ps_ap, wsb, cols, xt, n_free):
                """Emit the fp8 DR chain: raw kos (W8-hi broadcast over the
                (x_hi, x_lo) slot pair), then corrected kos (W_hi, W_lo)."""
                last = (NKO - 1, 1 if F1_RAW < NKO else 0)
                first = True
                for ko in range(NKO):
                    xpair = xt[:, ko, :, :]
                    if ko < F1_RAW:
                        wslot = wsb[:, ko, 0:1, cols].broadcast_to(
                            [P, 2, cols.stop - cols.start])
                        nc.tensor.matmul(
                            ps_ap, wslot, xpair,
                            start=first,
                            stop=(ko == NKO - 1 and F1_RAW == NKO),
                            perf_mode=DR)
                        first = False
                    else:
                        for t in range(2):
                            wslot = wsb[:, ko, t:t + 1, cols].broadcast_to(
                                [P, 2, cols.stop - cols.start])
                            nc.tensor.matmul(
                                ps_ap, wslot, xpair,
                                start=first,
                                stop=(ko == NKO - 1 and t == 1),
                                perf_mode=DR)
                            first = False

            def proj_groups(j):
                sl = slice(j * XC, (j + 1) * XC)
                xt = xt_tiles[j]
                qt_rings[j] = qT = pp0.tile([P, QL, XC], FP16, tag="qt",
                                            name="qTr")
                kR = pp0.tile([P, HL, XC], FP16, tag="kr", name="kRr")
                vR = pp0.tile([P, XC // P, HL * DH], FP16, tag="vr", name="vRr")
                kv_rings[j] = (kR, vR)
                groups = []

                def kq_group(qh, kq):
                    def emit():
                        ps = psum.tile([P, XC], F32, tag="aqk", bufs=2)
                        wsb = weffk_sb if kq == "k" else weffq_sb
                        proj_chain(ps[:], wsb,
                                   slice(qh * P, (qh + 1) * P), xt, XC)
                        if kq == "k":
                            rope_evict(ps[:], kR[:, qh, :], sl, kq, DESC_K)
                        else:
                            rope_evict(ps[:], qT[:, qh, :], sl, kq, DESC_Q)
                    return emit

                def v_group(ls):
                    def emit():
                        lt_idx = j * (XC // P) + ls
                        psv = psum.tile([P, 512], F32, tag="sm", bufs=2)
                        nv = HL * DH + HL
                        first = True
                        for ko in range(NKO):
                            xpair = xt[:, ko, :, ls * P:(ls + 1) * P]
                            for t in range(2):
                                wslot = weffv_sb[:, ko, t:t + 1, :].broadcast_to(
                                    [P, 2, nv])
                                nc.tensor.matmul(
                                    psv[:, :nv], xpair, wslot,
                                    start=first,
                                    stop=(ko == NKO - 1 and t == 1),
                                    perf_mode=DR)
                                first = False
                        if j >= 5:
                            nc.vector.tensor_scalar_mul(
                                vR[:, ls, :], psv[:, :HL * DH], DESC_V)
                        else:
                            nc.scalar.activation(
                                vR[:, ls, :], psv[:, :HL * DH],
                                mybir.ActivationFunctionType.Copy,
                                scale=DESC_V)
                        # lambda = sigmoid(v-chain cols [256:258] + blam)
                        e = tmpa.tile([P, HL], F32, tag="sig")
                        for hh in range(HL):
                            nc.scalar.activation(
                                e[:, hh:hh + 1],
                                psv[:, HL * DH + hh:HL * DH + hh + 1],
                                mybir.ActivationFunctionType.Exp,
                                scale=-DESC_V,
                                bias=nblam_sb[:, hh:hh + 1],
                            )
                        nc.vector.tensor_scalar_add(e[:], e[:], 1.0)
                        nc.vector.reciprocal(lam_sb[:, lt_idx, :], e[:])
                    return emit

                for h in range(HL):
                    groups.append(kq_group(h, "k"))
                for qh in range(QL):
                    groups.append(kq_group(qh, "q"))
                for ls in range(XC // P):
                    groups.append(v_group(ls))
                return groups

            def flush_k(j):
                kR, _ = kv_rings[j]
                sl = slice(j * XC, (j + 1) * XC)
                nc.vector.tensor_copy(kT_sb[:, :, sl], kR[:])

            def flush_v(j):
                _, vR = kv_rings.pop(j)
                nc.gpsimd.tensor_copy(
                    v_sb[:, 2 * j:2 * j + 2, :], vR[:])

            # Cross-iteration state: per j-block prob tiles, sums, attnT, pth.
            blk = {}

            # ---- score units for j-block: one closure per (h,p_,qi,ck) ----
            def score_units(j):
                qa, qb = 2 * j, 2 * j + 1
                ptiles, sums = {}, {}
                blk[j] = {"pt": ptiles, "sm": sums}
                units = []

                qT = qt_rings[j]

                def unit(h, p_, qi, qt, ck, nck):
                    def emit():
                        qh = 2 * h + p_
                        lk = (qt + 1) * P
                        if ck == 0:
                            ptiles[(h, p_, qi)] = pp.tile(
                                [P, L], FP16, tag=f"P{h}{p_}{qi}",
                                name=f"P{h}{p_}{qi}")
                            sums[(h, p_, qi)] = (
                                smp.tile([P, 2], F32, tag=f"s{h}{p_}{qi}",
                                         name=f"s{h}{p_}{qi}"), nck)
                        ptile = ptiles[(h, p_, qi)]
                        sm = sums[(h, p_, qi)][0]
                        cw = min(SC, lk - ck * SC)
                        sps = psum.tile([P, SC], F32, tag="sps", bufs=2)
                        for sb0 in range(0, cw, 512):
                            sw = min(512, cw - sb0)
                            nc.tensor.matmul(
                                sps[:, sb0:sb0 + sw],
                                qT[:, qh, (qt % 2) * P:(qt % 2 + 1) * P],
                                kT_sb[:, h, ck * SC + sb0:ck * SC + sb0 + sw],
                                start=True,
                                stop=True,
                            )
                        if ck * SC <= qt * P < ck * SC + cw:
                            off = qt * P - ck * SC
                            nc.vector.tensor_add(
                                sps[:, off:off + P], sps[:, off:off + P],
                                cmask[:])
                        nc.scalar.activation(
                            ptile[:, ck * SC:ck * SC + cw],
                            sps[:, :cw],
                            mybir.ActivationFunctionType.Exp,
                            scale=SCALE,
                            accum_out=sm[:, ck:ck + 1],
                        )
                    return emit

                h0_units = []
                h1_units = []
                for h in range(HL):
                    for qi, qt in enumerate((qa, qb)):
                        for p_ in range(2):
                            nck = ((qt + 1) * P + SC - 1) // SC
                            for ck in range(nck):
                                (h0_units if h == 0 else h1_units).append(
                                    unit(h, p_, qi, qt, ck, nck))
                return h0_units, h1_units

            # ---- combine + transpose for (j, h): DVE + sync queue ----
            def combine_tp(j, h, only_qi=None):
                qa, qb = 2 * j, 2 * j + 1
                ptiles, sums = blk[j]["pt"], blk[j]["sm"]
                for qi, qt in enumerate((qa, qb)):
                    if only_qi is not None and qi != only_qi:
                        continue
                    lk = (qt + 1) * P
                    invs = []
                    for p_ in range(2):
                        sm, nck = sums[(h, p_, qi)]
                        if nck > 1:
                            r = smp.tile([P, 1], F32, tag=f"r{h}{p_}{qi}")
                            nc.vector.reduce_sum(
                                r[:], sm[:, :nck], axis=mybir.AxisListType.X)
                        else:
                            r = sm[:, 0:1]
                        inv = smp.tile([P, 1], F32, tag=f"i{h}{p_}{qi}")
                        nc.vector.reciprocal(inv[:], r[:])
                        invs.append(inv)
                    s2 = smp.tile([P, 1], F32, tag=f"l{h}{qi}")
                    nc.vector.tensor_mul(
                        s2[:], invs[1][:], lam_sb[:, qt, h:h + 1])
                    p0, p1 = ptiles[(h, 0, qi)], ptiles[(h, 1, qi)]
                    nc.vector.tensor_scalar_mul(p1[:, :lk], p1[:, :lk], s2[:])
                    nc.vector.tensor_scalar_mul(p0[:, :lk], p0[:, :lk], invs[0][:])
                    nc.vector.tensor_sub(p0[:, :lk], p0[:, :lk], p1[:, :lk])
                if only_qi in (None, 0):
                    pth = ptp.tile([P, LT, 2 * P], FP16, tag="pt", name="pth")
                    blk[j][("pth", h)] = pth
                pth = blk[j][("pth", h)]
                for qi, qt in enumerate((qa, qb)):
                    if only_qi is not None and qi != only_qi:
                        continue
                    for c0 in range(0, qt + 1, 8):
                        c1 = min(c0 + 8, qt + 1)
                        nc.sync.dma_start_transpose(
                            pth[:, c0:c1, qi * P:(qi + 1) * P],
                            ptiles[(h, 0, qi)][:, c0 * P:c1 * P],
                        )

            # ---- PV for (j, h): PE + Act attnT evict ----
            def pv_h(j, h, only_qi=None):
                qa, qb = 2 * j, 2 * j + 1
                if "at" not in blk[j]:
                    blk[j]["at"] = atp.tile([P, HL, 2 * P], FP16, tag="at",
                                            name="attnT")
                attnT = blk[j]["at"]
                if only_qi is None:
                    pth = blk[j].pop(("pth", h))
                    pv = psum.tile([P, 512], F32, tag="sm", bufs=2)
                    for kc in range(qa + 1):
                        nc.tensor.matmul(
                            pv[:, :2 * P],
                            v_sb[:, kc, h * DH:(h + 1) * DH],
                            pth[:, kc, :],
                            start=(kc == 0),
                            stop=False,
                        )
                    nc.tensor.matmul(
                        pv[:, P:2 * P],
                        v_sb[:, qb, h * DH:(h + 1) * DH],
                        pth[:, qb, P:2 * P],
                        start=False,
                        stop=True,
                    )
                    nc.vector.tensor_copy(attnT[:, h, :], pv[:, :2 * P])
                else:
                    qt = (qa, qb)[only_qi]
                    pth = blk[j][("pth", h)]
                    if only_qi == 1:
                        blk[j].pop(("pth", h))
                    pv = psum.tile([P, 512], F32, tag="sm", bufs=2)
                    for kc in range(qt + 1):
                        nc.tensor.matmul(
                            pv[:, :P],
                            v_sb[:, kc, h * DH:(h + 1) * DH],
                            pth[:, kc, only_qi * P:(only_qi + 1) * P],
                            start=(kc == 0),
                            stop=(kc == qt),
                        )
                    nc.vector.tensor_copy(
                        attnT[:, h, only_qi * P:(only_qi + 1) * P], pv[:, :P])

            # ---- W_out for block j, one qt, one 1024-half: PE + evict ----
            def wout_half(j, qi, half):
                qa, qb = 2 * j, 2 * j + 1
                qt = (qa, qb)[qi]
                attnT = blk[j]["at"]
                key = ("osb", qi)
                if key not in blk[j]:
                    blk[j][key] = outp.tile([P, D], FP16, tag=f"osb{qi}",
                                            name=f"osb{qi}")
                osb = blk[j][key]
                last = j == NC_CHUNKS - 1
                po = psum.tile([P, SC], F32, tag="sps", bufs=2)
                for nb in range(2):
                    for h in range(HL):
                        nc.tensor.matmul(
                            po[:, nb * 512:(nb + 1) * 512],
                            attnT[:, h, qi * P:(qi + 1) * P],
                            wout_sb[:, h,
                                    (2 * half + nb) * 512:
                                    (2 * half + nb + 1) * 512],
                            start=(h == 0),
                            stop=(h == HL - 1),
                        )
                    if last:
                        # quarter-granular evict+DMA: short drain
                        c0 = (2 * half + nb) * 512
                        if nb == 0:
                            nc.vector.tensor_copy(
                                osb[:, c0:c0 + 512], po[:, :512])
                        else:
                            nc.scalar.activation(
                                osb[:, c0:c0 + 512], po[:, 512:],
                                mybir.ActivationFunctionType.Copy)
                        nc.sync.dma_start(
                            y[qt * P:(qt + 1) * P, c0:c0 + 512],
                            osb[:, c0:c0 + 512])
                if not last:
                    if half == 0:
                        nc.vector.tensor_copy(
                            osb[:, half * SC:(half + 1) * SC], po[:])
                    else:
                        nc.scalar.activation(
                            osb[:, half * SC:(half + 1) * SC], po[:],
                            mybir.ActivationFunctionType.Copy)
                        nc.gpsimd.dma_start(
                            y[qt * P:(qt + 1) * P, :], osb[:])
                        blk[j].pop(key)

            # ---- half-block staggered emission ----
            # iter j: proj(j) woven with block(j-1) h1 scores (early) and
            # block(j) h0 scores (late); block(j-1) combine/tp/PV/W_out all
            # complete within iter j.
            # h1-pull: how many of block j's h1 units run inside iter j
            # (after its q2/q3 ropes) instead of deferring to iter j+1 —
            # smooths the Act-bound tail.
            PULL = kn["pull"]
            RATIO = kn.get("ratio", 1.0)
            pend_h1 = {}
            for j in range(NC_CHUNKS + 1):
                a = j if j < NC_CHUNKS else None
                s = j - 1 if j >= 1 else None          # writeback block

                if a is not None and j > 0:
                    xt_fetch(j + 1)
                pg = proj_groups(a) if a is not None else []
                pulled = []
                if a is not None:
                    h0u, h1u = score_units(a)
                    if j == 0:
                        # chunk 0: weights/tables still streaming in; defer
                        # everything to iter 1
                        pend_h1[a] = h0u + h1u
                        h0u = []
                    else:
                        np_ = PULL.get(a, 0)
                        pulled = h1u[:np_]
                        pend_h1[a] = h1u[np_:]
                else:
                    h0u = []
                eu = pend_h1.pop(s, []) if s is not None else []

                stream = []
                state = {"e": 0, "l": 0, "p": 0}

                def take(lst, key, k):
                    for _ in range(k):
                        if state[key] < len(lst):
                            stream.append(lst[state[key]])
                            state[key] += 1

                npg = len(pg)
                # pg order: k0 k1 q0 q1 q2 q3 v0 v1
                for gi in range(max(npg, 1)):
                    if gi == (4 if s == 0 else 2) and s is not None:
                        stream.append(lambda s=s: combine_tp(s, 0))
                    if pg:
                        stream.append(pg.pop(0))
                    if gi == 3 and a is not None:
                        stream.append(lambda a=a: flush_k(a))
                    if gi in kn["eu_gis"]:
                        take(eu, "e",
                             (len(eu) + len(kn["eu_gis"]) - 1)
                             // len(kn["eu_gis"]))
                    if gi == 7 and s is not None:
                        take(eu, "e", len(eu))
                        stream.append(lambda s=s: pv_h(s, 0))
                    if gi >= kn["h0_gi"] and j > 0:
                        # h0 units read only the q0/q1 ropes (pg2/pg3)
                        take(h0u, "l", kn["h0_rate"])
                    if gi >= kn["pull_gi"]:
                        # pulled h1 units read the q2/q3 ropes (pg4/pg5)
                        take(pulled, "p", kn["pull_rate"])
                    if gi == 6 and s is not None:
                        stream.append(lambda s=s: combine_tp(s, 1))
                if s is not None:
                    if npg == 0:
                        # epilogue: combine(s,0,*) and (s,1,0) ran in iter s;
                        # pv(1,0) late — its transposes are last in the queue
                        stream.append(lambda s=s: pv_h(s, 0, 0))
                        stream.append(lambda s=s: pv_h(s, 0, 1))
                        stream.append(lambda s=s: combine_tp(s, 1, 1))
                        stream.append(lambda s=s: pv_h(s, 1, 0))
                        stream.append(lambda s=s: wout_half(s, 0, 0))
                        stream.append(lambda s=s: wout_half(s, 0, 1))
                        stream.append(lambda s=s: pv_h(s, 1, 1))
                        stream.append(lambda s=s: wout_half(s, 1, 0))
                        stream.append(lambda s=s: wout_half(s, 1, 1))
                    else:
                        take(eu, "e", len(eu))
                        stream.append(lambda s=s: pv_h(s, 1))
                        take(h0u, "l", 2)
                        stream.append(lambda s=s: wout_half(s, 0, 0))
                        take(h0u, "l", 1)
                        stream.append(lambda s=s: wout_half(s, 0, 1))
                        take(h0u, "l", 1)
                        stream.append(lambda s=s: wout_half(s, 1, 0))
                        take(h0u, "l", 1)
                        stream.append(lambda s=s: wout_half(s, 1, 1))
                take(h0u, "l", len(h0u))
                take(pulled, "p", len(pulled))
                if a is not None:
                    stream.append(lambda a=a: flush_v(a))
                if a == NC_CHUNKS - 1:
                    # start the last block's combines + transposes now so
                    # the epilogue's PV has its pth ready
                    stream.append(lambda a=a: combine_tp(a, 0, 0))
                    stream.append(lambda a=a: combine_tp(a, 0, 1))
                    stream.append(lambda a=a: combine_tp(a, 1, 0))
                if j == 0:
                    stream.insert(len(stream) * 3 // 4, lambda: xt_fetch(1))
                    stream.insert(len(stream) // 2, fetch_wout)

                for emit in stream:
                    emit()
                if a is not None:
                    xt_tiles.pop(a, None)

    nc.compile()
    return nc


_NC = None


def _get_nc():
    global _NC
    if _NC is None:
        _NC = build_nc()
    return _NC


def _rope_tables():
    inv_freq = 1.0 / (10000.0 ** (np.arange(0, DH, 2, dtype=np.float32) / DH))
    t = np.arange(L, dtype=np.float32)
    freqs = np.outer(t, inv_freq)                    # [L, DH/2]
    emb = np.concatenate([freqs, freqs], axis=-1)    # [L, DH]
    cos = np.cos(emb).astype(np.float32)
    sin = np.sin(emb).astype(np.float32)
    sign = np.where(np.arange(DH) < DH // 2, -1.0, 1.0).astype(np.float32)
    cosT = np.ascontiguousarray(cos.T)               # [DH, L]
    sinTs = np.ascontiguousarray(sin.T * sign[:, None])
    return cosT, sinTs


def _hp(a):
    return np.ascontiguousarray(np.asarray(a, dtype=np.float32)).astype(
        np.float16
    )


def _fp8_pair(a, scale):
    """[..., n] -> [..., 2, n]: hi = e4m3(a*scale), lo = e4m3(a*scale - hi)."""
    s = np.asarray(a, dtype=np.float32) * scale
    hi = s.astype(E4M3)
    lo = (s - hi.astype(np.float32)).astype(E4M3)
    return np.ascontiguousarray(np.stack([hi, lo], axis=-2))


def prepare_in_maps(x, W_DKV, W_UK, W_UV, W_DQ, W_UQ, W_lam, b_lam, W_out):
    x = np.asarray(x, dtype=np.float32)
    W_DKV = np.asarray(W_DKV, dtype=np.float32)
    W_UK = np.asarray(W_UK, dtype=np.float32)
    W_UV = np.asarray(W_UV, dtype=np.float32)
    W_DQ = np.asarray(W_DQ, dtype=np.float32)
    W_UQ = np.asarray(W_UQ, dtype=np.float32)
    W_lam = np.asarray(W_lam, dtype=np.float32)
    b_lam = np.asarray(b_lam, dtype=np.float32)
    W_out = np.asarray(W_out, dtype=np.float32)

    # Host-side low-rank fold (fp32 BLAS)
    Weffq = W_DQ @ W_UQ                              # [D, 2*NH*DH]
    Weffk = W_DKV @ W_UK                             # [D, NH*DH]
    Weffv = W_DKV @ W_UV

    x8 = _fp8_pair(x[0].T, SX)                       # [D, 2, L]
    x8 = np.ascontiguousarray(                       # [D, NC, 2, XC]
        x8.reshape(D, 2, NC_CHUNKS, XC).transpose(0, 2, 1, 3))
    cosT, sinTs = _rope_tables()
    cosT_hp, sinTs_hp = _hp(cosT), _hp(sinTs)

    in_maps = []
    for c in range(NCORES):
        nblam_ = np.ascontiguousarray(
            np.broadcast_to(-b_lam[2 * c:2 * c + 2][None, :], (P, HL))
        ).astype(np.float32)
        in_maps.append({
            "x8T": x8,
            "weffq": _fp8_pair(Weffq[:, 4 * c * 128:(4 * c + 4) * 128], SWQ),
            "weffk": _fp8_pair(Weffk[:, c * 256:(c + 1) * 256], SWK),
            "weffv": _fp8_pair(np.concatenate(
                [Weffv[:, c * 256:(c + 1) * 256],
                 W_lam[:, 2 * c:2 * c + 2]], axis=1), SWV),
            "nblam": nblam_,
            "wout": _hp(W_out[c * 256:(c + 1) * 256, :]),
            "cosT": cosT_hp,
            "sinTs": sinTs_hp,
        })
    return in_maps


def kernel(x, W_DKV, W_UK, W_UV, W_DQ, W_UQ, W_lam, b_lam, W_out):
    in_maps = prepare_in_maps(
        x, W_DKV, W_UK, W_UV, W_DQ, W_UQ, W_lam, b_lam, W_out)

    from concourse.bass_utils import run_bass_kernel_spmd
    nc = _get_nc()
    res = run_bass_kernel_spmd(nc, in_maps, core_ids=list(range(NCORES)))
    y = np.zeros((L, D), dtype=np.float32)
    for c in range(NCORES):
        y += np.asarray(res.results[c]["y"], dtype=np.float32)
    return y.reshape(1, L, D)
